# revision 9
# baseline (speedup 1.0000x reference)
"""AffineTransformLayer as a Trainium2 Bass kernel, SPMD over 8 NeuronCores.

Pair-gather architecture over a dual-parity bf16 image.

Measured cost laws (hardware, this device):
- dma_gather costs ~7.5ns per idx SLOT on one Q7 core pair; 4 queue-pairs
  run concurrently; cost is independent of elem size. So fetch PAIRS of
  output pixels per slot (163840 slots/core vs 327680 for per-pixel).
- The DMA engines run ~11-15GB/s each on small-descriptor gathers and were
  ~80% busy moving 84MB/core in f32 - the byte volume co-limits. Storing
  the image in bf16 halves bytes: each 256B descriptor covers 2 pixels.
- DVE: big tensor_tensor ops with a stride-0 broadcast weight AP plus a
  bf16 pairwise max tree beat per-column scalar_tensor_tensor by ~2.5x.

Image layout ("dual parity"): unit u = 256B = 2 bf16 pixels. Units
0..32767 hold pixel pairs (2k, 2k+1); units 32768..65535 hold (2k+1, 2k+2)
(one-pixel shift), so a pair anchored at ANY pixel parity is one unit.
Unit 65536 is a -1e30 sentinel pair; the int16 idx window (base unit
32769) spans units 1..65536. B-tail pad (unit 65535, sub 1) holds pixel
0's value to make pixel-0 fixups addressable.

Correctness of max over transforms:
- main pass: "clean" pairs (sources consecutive, or any pixel invalid)
  contribute w*x[src] with w=0 for invalid pixels (reference gives 0);
  "dirty" pairs (both valid, sources not consecutive, or anchor
  unreachable, or last-slot-of-instruction) fetch the sentinel pair so
  they never win the on-device max tree.
- fixup pass: each dirty pair's two pixels are gathered via their units,
  weighted on device (weight on the right sub-pixel, 0 on the other), and
  streamed out; the host max-merges these products into the main result at
  host-known positions. Exact coverage: every (n, pixel) entry is in
  exactly one of the two passes.
- the last slot of every 1024-idx instruction is forced dirty (sentinel,
  idx 32767 >= 0) because the ucode drops TRAILING negative idxs, which
  would break the DMA-completion semaphore count.

The fixup stream is provisioned (padded) per call to the worst core's
need, rounded to 8 instructions; programs are compiled per provision size
and cached. Output and fixup products return in bf16 (~0.2% rounding;
harness gate is 2e-2) and the host converts/unshuffles.
"""

import sys

sys.path.insert(0, "/opt/trn_rl_repo")

import numpy as np

B, H, W, C = 4, 256, 256, 64
N = 10
HM, WM = 64, 64
NPIX = H * W
HALF = NPIX // 2  # output pixels per core
ROWS = H // 2
NBLK = 8  # main blocks per core
BLKPX = HALF // NBLK  # 4096 output px per block
G = 4  # output pixels per main gather slot (quad)
PAIRS = BLKPX // G  # 1024 quad slots per main instruction
SENT_ROW = 65536  # first sentinel row in xbs
MAIN_BASE = 32769  # window base pixel for main pass
FIX_BASE = 32768
SENT_IDX = SENT_ROW - MAIN_BASE  # 32767
FIXSLOT = 1024  # px slots per fixup instruction

_programs = {}


def _host_index_weights(transforms, mask):
    """Reference coordinate math, op-for-op in float32.

    Returns flat src [B,N,NPIX] int64, weight [B,N,NPIX] f32 (0 where
    invalid), valid [B,N,NPIX] bool.
    """
    one = np.float32(1.0)
    p = (transforms.astype(np.float32) / one).astype(np.float32)
    X = np.arange(W, dtype=np.float32)[None, None, None, :]
    Y = np.arange(H, dtype=np.float32)[None, None, :, None]

    def coef(i):
        return p[:, :, i][:, :, None, None]

    k = coef(6) * X + coef(7) * Y + one
    in_x = ((coef(0) * X + coef(1) * Y) + coef(2)) / k
    in_y = ((coef(3) * X + coef(4) * Y) + coef(5)) / k
    ix = np.round(in_x).astype(np.int32)
    iy = np.round(in_y).astype(np.int32)
    valid = (ix >= 0) & (ix < W) & (iy >= 0) & (iy < H)
    ixc = np.clip(ix, 0, W - 1)
    iyc = np.clip(iy, 0, H - 1)
    flat = (iyc.astype(np.int64) * W + ixc).reshape(B, N, NPIX)

    ys = np.minimum(np.arange(H) * HM // H, HM - 1)
    xs = np.minimum(np.arange(W) * WM // W, WM - 1)
    m_up = mask[:, :, ys][:, :, :, xs]
    wgt = np.where(valid, m_up, np.float32(0.0)).astype(np.float32).reshape(B, N, NPIX)
    return flat, wgt, valid.reshape(B, N, NPIX)


def _build_program(nfix):
    import os

    import concourse.bass as bass
    import concourse.tile as tile
    from concourse import bacc, mybir
    from concourse.ap import AP

    skip_dve = os.environ.get("K_SKIP_DVE") == "1"
    no_fix = os.environ.get("K_NO_FIX") == "1"
    nqueues = int(os.environ.get("K_QUEUES", "4"))
    nc = bacc.Bacc(
        "TRN2",
        target_bir_lowering=False,
        debug=False,
        num_swdge_queues=nqueues,
        dynamic_dma_scratch_size=int(os.environ.get("K_SCRATCH", "32768")),
    )
    xbs = nc.dram_tensor(
        "xbs", [NPIX // 2 * 2 + 2, 2 * C], mybir.dt.bfloat16, kind="ExternalInput"
    ).ap()
    midx = nc.dram_tensor(
        "midx", [NBLK, 128, N * (PAIRS // 16)], mybir.dt.int16, kind="ExternalInput"
    ).ap()
    mw = nc.dram_tensor(
        "mw", [NBLK, 128, N * (PAIRS // 128) * G], mybir.dt.bfloat16,
        kind="ExternalInput",
    ).ap()
    outd = nc.dram_tensor(
        "outd", [NBLK, 128, PAIRS // 128, G, C], mybir.dt.bfloat16,
        kind="ExternalOutput",
    ).ap()
    if nfix:
        fidx = nc.dram_tensor(
            "fidx", [nfix, 128, FIXSLOT // 16], mybir.dt.int16, kind="ExternalInput"
        ).ap()
        fw = nc.dram_tensor(
            "fw", [nfix, 128, FIXSLOT // 128, 2], mybir.dt.bfloat16,
            kind="ExternalInput",
        ).ap()
        fixout = nc.dram_tensor(
            "fixout", [nfix, 128, FIXSLOT // 128, 2, C], mybir.dt.bfloat16,
            kind="ExternalOutput",
        ).ap()

    # gather window over 256B units (2 bf16 px): unit MAIN_BASE+k,
    # k in [-32768, 32767] -> units 1..65536 (unit 65536 = sentinel pair).
    # main window: overlapping 512B (2-unit) windows at 1-unit stride
    srcw = AP(
        xbs.tensor, MAIN_BASE * 2 * C, [[2 * C, NPIX // 2 * 2 - MAIN_BASE + 1], [1, 4 * C]]
    )
    srcf = xbs[FIX_BASE:, :]

    MCOL = PAIRS // 128  # 8
    FCOL = FIXSLOT // 128  # 8
    IDXF = PAIRS // 16  # 64
    nfix_per_blk = [nfix // NBLK + (1 if b < nfix % NBLK else 0) for b in range(NBLK)]

    with tile.TileContext(nc) as tc:
        nreg = nc.gpsimd.to_reg(PAIRS)
        nreg512 = nc.gpsimd.to_reg(PAIRS // 2)
        qctr = [0]
        fctr = [0]
        with (
            tc.tile_pool(name="ip", bufs=6) as ip,
            tc.tile_pool(name="wp", bufs=6) as wp,
            tc.tile_pool(name="gp", bufs=14) as gp,
            tc.tile_pool(name="tp", bufs=10) as tp,
            tc.tile_pool(name="fip", bufs=8) as fip,
            tc.tile_pool(name="fwp", bufs=8) as fwp,
            tc.tile_pool(name="fgp", bufs=8) as fgp,
            tc.tile_pool(name="fpp", bufs=6) as fpp,
        ):

            def emit_fix(f):
                it = fip.tile([128, FIXSLOT // 16], mybir.dt.int16, tag="fi")
                nc.scalar.dma_start(it[:], fidx[f, :, :])
                wt = fwp.tile([128, FCOL, 2], mybir.dt.bfloat16, tag="fw")
                nc.scalar.dma_start(wt[:], fw[f, :, :, :])
                g = fgp.tile([128, FCOL, 2, C], mybir.dt.bfloat16, tag="fg")
                nc.gpsimd.dma_gather(
                    g[:, :, :, :].rearrange("p c s e -> p c (s e)"),
                    srcf,
                    it[:, :],
                    num_idxs=FIXSLOT,
                    num_idxs_reg=nreg,
                    elem_size=2 * C,
                    queue_num=qctr[0] % nqueues,
                )
                qctr[0] += 1
                pf = fpp.tile([128, FCOL, 2, C], mybir.dt.bfloat16, tag="pf")
                nc.vector.tensor_tensor(
                    pf[:, :, :, :],
                    g[:, :, :, :],
                    wt[:, :, :].broadcast_to([128, FCOL, 2, C]),
                    op=mybir.AluOpType.mult,
                )
                nc.sync.dma_start(fixout[f, :, :, :, :], pf[:, :, :, :])

            for blk in range(NBLK):
                it = ip.tile([128, N * IDXF], mybir.dt.int16, tag="mi")
                nc.scalar.dma_start(it[:], midx[blk, :, :])
                wt = wp.tile([128, N, MCOL, G], mybir.dt.bfloat16, tag="mw")
                nc.scalar.dma_start(
                    wt[:].rearrange("p n c s -> p (n c s)"), mw[blk, :, :]
                )
                # per-n masked products in bf16, then a 9-op pairwise max tree
                gs_last = [None]
                pend = []  # partial maxes awaiting pairing, with tree level
                for n in range(N):
                    g = gp.tile([128, MCOL, G, C], mybir.dt.bfloat16, tag="g")
                    nc.gpsimd.dma_gather(
                        g[:, :, :, :].rearrange("p c s e -> p c (s e)"),
                        srcw,
                        it[:, n * IDXF : (n + 1) * IDXF],
                        num_idxs=PAIRS,
                        num_idxs_reg=nreg,
                        elem_size=G * C,
                        elem_step=2 * C,
                        queue_num=qctr[0] % nqueues,
                    )
                    qctr[0] += 1
                    if skip_dve:
                        gs_last[0] = g
                        continue
                    t = tp.tile([128, MCOL, G, C], mybir.dt.bfloat16, tag="t")
                    nc.vector.tensor_tensor(
                        t[:, :, :, :],
                        g[:, :, :, :],
                        wt[:, n, :, :].broadcast_to([128, MCOL, G, C]),
                        op=mybir.AluOpType.mult,
                    )
                    pend.append((0, t))
                    # greedily combine equal-level partials (keeps tiles hot)
                    while len(pend) >= 2 and pend[-1][0] == pend[-2][0]:
                        l1, a = pend.pop()
                        l0, bb = pend.pop()
                        m = tp.tile([128, MCOL, G, C], mybir.dt.bfloat16, tag="t")
                        nc.vector.tensor_tensor(
                            m[:, :, :, :], a[:, :, :, :], bb[:, :, :, :],
                            op=mybir.AluOpType.max,
                        )
                        pend.append((l0 + 1, m))
                while len(pend) >= 2:
                    _, a = pend.pop()
                    _, bb = pend.pop()
                    m = tp.tile([128, MCOL, G, C], mybir.dt.bfloat16, tag="t")
                    nc.vector.tensor_tensor(
                        m[:, :, :, :], a[:, :, :, :], bb[:, :, :, :],
                        op=mybir.AluOpType.max,
                    )
                    pend.append((99, m))
                if skip_dve:
                    acc = tp.tile([128, MCOL, G, C], mybir.dt.bfloat16, tag="t")
                    nc.vector.tensor_tensor(
                        acc[:, :, :, :],
                        gs_last[0][:, :, :, :],
                        gs_last[0][:, :, :, :],
                        op=mybir.AluOpType.max,
                    )
                else:
                    acc = pend[0][1]
                nc.sync.dma_start(outd[blk, :, :, :, :], acc[:, :, :, :])
                if not no_fix:
                    for _ in range(nfix_per_blk[blk]):
                        emit_fix(fctr[0])
                        fctr[0] += 1
    nc.compile()
    return nc


def _get_program(nfix):
    if nfix not in _programs:
        _programs[nfix] = _build_program(nfix)
    return _programs[nfix]


def _wrap_idx(idx_slots):
    """[..., S] slot-ordered int16 -> [..., 128, S//16] wrapped+replicated."""
    S = idx_slots.shape[-1]
    q = np.arange(S)
    out = np.zeros(idx_slots.shape[:-1] + (128, S // 16), np.int16)
    for g_ in range(8):
        out[..., 16 * g_ + (q % 16), q // 16] = idx_slots
    return out


def kernel(x, transforms, mask):
    from concourse.bass_utils import run_bass_kernel_spmd
    import ml_dtypes

    bf16 = ml_dtypes.bfloat16
    x = np.asarray(x, dtype=np.float32)
    transforms = np.asarray(transforms, dtype=np.float32)
    mask = np.asarray(mask, dtype=np.float32)

    flat, wgt, valid = _host_index_weights(transforms, mask)

    SENT = np.float32(-1e30)
    q = np.arange(PAIRS)

    def unit_of(px):
        # pixel index -> 256B unit in the dual-parity bf16 image
        return np.where(px % 2 == 0, px // 2, 32768 + (px - 1) // 2)

    per_core = []
    max_fix = 0
    for core in range(8):
        b, half = divmod(core, 2)
        sl = slice(half * HALF, (half + 1) * HALF)
        s = flat[b, :, sl]  # [N, HALF]
        w = wgt[b, :, sl]
        v = valid[b, :, sl]

        sq = s.reshape(N, -1, G)
        vq = v.reshape(N, -1, G)
        wq = w.reshape(N, -1, G)
        off = np.arange(G)
        base = sq - off
        anchor = np.where(vq, base, -(10**9)).max(axis=-1)
        anyv = vq.any(axis=-1)
        agree = ((base == anchor[..., None]) | ~vq).all(axis=-1)
        addr_ok = (anchor >= 2) & (anchor <= NPIX - G)
        clean = np.where(anyv, agree & addr_ok, True)
        dirty = ~clean
        # trailing-negative guard: last slot of each instruction -> sentinel
        dirty[:, PAIRS - 1 :: PAIRS] = True

        aunit = unit_of(np.maximum(anchor, 2))
        idx = np.where(
            dirty, SENT_IDX, np.where(anyv, aunit - MAIN_BASE, 0)
        ).astype(np.int16)
        mwq = np.where(dirty[..., None], 1.0, np.where(vq, wq, 0.0)).astype(
            np.float32
        )

        # fixup: dirty quads decompose into two pairs; a clean pair is one
        # unit-slot covering both pixels, a dirty pair two single-px slots.
        dn, dq = np.nonzero(dirty)
        ju, j0l, j1l, w0l, w1l = [], [], [], [], []
        for p_ in (0, 1):
            s0 = sq[dn, dq, 2 * p_]
            s1 = sq[dn, dq, 2 * p_ + 1]
            v0 = vq[dn, dq, 2 * p_]
            v1 = vq[dn, dq, 2 * p_ + 1]
            w0 = wq[dn, dq, 2 * p_]
            w1 = wq[dn, dq, 2 * p_ + 1]
            jj0 = (dq * G + 2 * p_).astype(np.int64)
            both = v0 & v1
            ap_ = np.where(v0, s0, s1 - 1)
            pclean = np.where(both, s1 == s0 + 1, True) & (ap_ >= 0)
            pc = pclean
            ju.append(unit_of(np.maximum(ap_[pc], 0)))
            j0l.append(jj0[pc])
            j1l.append(jj0[pc] + 1)
            w0l.append(np.where(v0[pc], w0[pc], 0.0))
            w1l.append(np.where(v1[pc], w1[pc], 0.0))
            pd = ~pclean
            npd = int(pd.sum())
            for sx, vx, wx, jx in (
                (s0, v0, w0, jj0),
                (s1, v1, w1, jj0 + 1),
            ):
                ju.append(unit_of(np.clip(sx[pd], 0, NPIX - 1)))
                j0l.append(jx[pd])
                j1l.append(np.full(npd, -1, np.int64))
                w0l.append(np.where(vx[pd], wx[pd], 0.0))
                w1l.append(np.zeros(npd, np.float32))
        fix_unit = np.concatenate(ju)
        fix_j0 = np.concatenate(j0l)
        fix_j1 = np.concatenate(j1l)
        fix_w0 = np.concatenate(w0l).astype(np.float32)
        fix_w1 = np.concatenate(w1l).astype(np.float32)

        per_core.append((b, idx, mwq, fix_unit, fix_j0, fix_j1, fix_w0, fix_w1))
        nfix_c = -(-len(fix_unit) // (FIXSLOT - 1))
        max_fix = max(max_fix, nfix_c)

    nfix = max(8, -(-max_fix // 8) * 8) if max_fix else 8

    in_maps = []
    fix_meta = []
    for core in range(8):
        b, idx, mwq, fix_unit, fix_j0, fix_j1, fix_w0, fix_w1 = per_core[core]

        idx_b = idx.reshape(N, NBLK, PAIRS)
        miw = _wrap_idx(idx_b)
        miw = np.ascontiguousarray(miw.transpose(1, 2, 0, 3)).reshape(
            NBLK, 128, N * (PAIRS // 16)
        )
        wl = np.zeros((N, NBLK, 128, PAIRS // 128, G), np.float32)
        wl[:, :, q % 128, q // 128, :] = mwq.reshape(N, NBLK, PAIRS, G)[:, :, q, :]
        mwl = (
            np.ascontiguousarray(wl.transpose(1, 2, 0, 3, 4))
            .reshape(NBLK, 128, N * (PAIRS // 128) * G)
            .astype(bf16)
        )

        # fixup: 1023 real slots/instruction; window based at unit FIX_BASE
        # so every pixel (incl. 0) is sub-0 of its unit. Pads use an odd
        # pixel (unit >= 32768 -> idx >= 0) so the list never ends negative.
        k = len(fix_unit)
        real = FIXSLOT - 1
        tot = nfix * real
        funit = np.full(tot, FIX_BASE + 1, np.int64)  # pad: idx stays >= 0
        fw0 = np.zeros(tot, np.float32)
        fw1 = np.zeros(tot, np.float32)
        funit[:k] = fix_unit
        fw0[:k] = fix_w0
        fw1[:k] = fix_w1
        funit = np.concatenate(
            [funit.reshape(nfix, real), np.full((nfix, 1), FIX_BASE + 1, np.int64)],
            axis=1,
        )
        fw0 = np.concatenate(
            [fw0.reshape(nfix, real), np.zeros((nfix, 1), np.float32)], axis=1
        )
        fw1 = np.concatenate(
            [fw1.reshape(nfix, real), np.zeros((nfix, 1), np.float32)], axis=1
        )
        fidx16 = (funit - FIX_BASE).astype(np.int16)
        fiw = _wrap_idx(fidx16)
        qf = np.arange(FIXSLOT)
        fwl2 = np.zeros((nfix, 128, FIXSLOT // 128, 2), np.float32)
        fwl2[:, qf % 128, qf // 128, 0] = fw0[:, qf]
        fwl2[:, qf % 128, qf // 128, 1] = fw1[:, qf]
        fwl2 = fwl2.astype(bf16)

        # dual-parity bf16 image + sentinel unit
        xb = x[b].reshape(NPIX, C).astype(bf16)
        A = xb.reshape(NPIX // 2, 2 * C)
        Bsh = np.empty((NPIX // 2, 2, C), bf16)
        Bsh[:, 0, :] = xb[1:NPIX:2]
        Bsh[:-1, 1, :] = xb[2:NPIX:2]
        Bsh[-1, 1, :] = xb[0]  # pad = pixel 0 (serves s==0 fixups)
        sent = np.full((2, 2 * C), SENT, np.float32).astype(bf16)
        xbs = np.concatenate(
            [A, Bsh.reshape(NPIX // 2, 2 * C), sent], axis=0
        )

        in_maps.append(
            {
                "xbs": xbs,
                "midx": miw,
                "mw": mwl,
                "fidx": fiw,
                "fw": fwl2,
            }
        )
        fix_meta.append((fix_j0, fix_j1, k))

    nc = _get_program(nfix)
    res = run_bass_kernel_spmd(nc, in_maps, list(range(8)))

    out = np.empty((B, H, W, C), np.float32)
    qf = np.arange(FIXSLOT)
    for core in range(8):
        b, half = divmod(core, 2)
        o = res.results[core]["outd"]  # [NBLK,128,8,2,C] bf16
        pair_vals = o[:, q % 128, q // 128, :, :]
        half_px = pair_vals.reshape(HALF, C).astype(np.float32)

        fix_j0, fix_j1, k = fix_meta[core]
        if k:
            fo = res.results[core]["fixout"]  # [nfix,128,8,2,C]
            fv = fo[:, qf % 128, qf // 128, :, :]  # [nfix, 1024, 2, C]
            fv = fv[:, : FIXSLOT - 1, :, :].reshape(-1, 2, C)[:k]
            np.maximum.at(half_px, fix_j0, fv[:, 0, :].astype(np.float32))
            pm = fix_j1 >= 0
            if pm.any():
                np.maximum.at(
                    half_px, fix_j1[pm], fv[pm, 1, :].astype(np.float32)
                )

        out[b, half * ROWS : (half + 1) * ROWS] = half_px.reshape(ROWS, W, C)
    return out


# revision 12
# speedup vs baseline: 1.2838x; 1.2838x over previous
"""AffineTransformLayer as a Trainium2 Bass kernel, SPMD over 8 NeuronCores.

Pair-gather architecture over a dual-parity bf16 image.

Measured cost laws (hardware, this device):
- dma_gather costs ~7.5ns per idx SLOT on one Q7 core pair; 4 queue-pairs
  run concurrently; cost is independent of elem size. So fetch PAIRS of
  output pixels per slot (163840 slots/core vs 327680 for per-pixel).
- The DMA engines run ~11-15GB/s each on small-descriptor gathers and were
  ~80% busy moving 84MB/core in f32 - the byte volume co-limits. Storing
  the image in bf16 halves bytes: each 256B descriptor covers 2 pixels.
- DVE: big tensor_tensor ops with a stride-0 broadcast weight AP plus a
  bf16 pairwise max tree beat per-column scalar_tensor_tensor by ~2.5x.

Image layout ("dual parity"): unit u = 256B = 2 bf16 pixels. Units
0..32767 hold pixel pairs (2k, 2k+1); units 32768..65535 hold (2k+1, 2k+2)
(one-pixel shift), so a pair anchored at ANY pixel parity is one unit.
Unit 65536 is a -1e30 sentinel pair; the int16 idx window (base unit
32769) spans units 1..65536. B-tail pad (unit 65535, sub 1) holds pixel
0's value to make pixel-0 fixups addressable.

Correctness of max over transforms:
- main pass: "clean" pairs (sources consecutive, or any pixel invalid)
  contribute w*x[src] with w=0 for invalid pixels (reference gives 0);
  "dirty" pairs (both valid, sources not consecutive, or anchor
  unreachable, or last-slot-of-instruction) fetch the sentinel pair so
  they never win the on-device max tree.
- fixup pass: each dirty pair's two pixels are gathered via their units,
  weighted on device (weight on the right sub-pixel, 0 on the other), and
  streamed out; the host max-merges these products into the main result at
  host-known positions. Exact coverage: every (n, pixel) entry is in
  exactly one of the two passes.
- the last slot of every 1024-idx instruction is forced dirty (sentinel,
  idx 32767 >= 0) because the ucode drops TRAILING negative idxs, which
  would break the DMA-completion semaphore count.

The fixup stream is provisioned (padded) per call to the worst core's
need, rounded to 8 instructions; programs are compiled per provision size
and cached. Output and fixup products return in bf16 (~0.2% rounding;
harness gate is 2e-2) and the host converts/unshuffles.
"""

import sys

sys.path.insert(0, "/opt/trn_rl_repo")

import numpy as np

B, H, W, C = 4, 256, 256, 64
N = 10
HM, WM = 64, 64
NPIX = H * W
HALF = NPIX // 2  # output pixels per core
ROWS = H // 2
NBLK = 16  # main blocks per core
BLKPX = HALF // NBLK  # 2048 output px per block
PAIRS = BLKPX // 2  # 1024 pair slots per main instruction
SENT_ROW = 65536  # first sentinel row in xbs
MAIN_BASE = 32769  # window base pixel for main pass
FIX_BASE = 32768
SENT_IDX = SENT_ROW - MAIN_BASE  # 32767
FIXSLOT = 1024  # px slots per fixup instruction

_programs = {}


def _host_index_weights(transforms, mask):
    """Reference coordinate math, op-for-op in float32.

    Returns flat src [B,N,NPIX] int64, weight [B,N,NPIX] f32 (0 where
    invalid), valid [B,N,NPIX] bool.
    """
    one = np.float32(1.0)
    p = (transforms.astype(np.float32) / one).astype(np.float32)
    X = np.arange(W, dtype=np.float32)[None, None, None, :]
    Y = np.arange(H, dtype=np.float32)[None, None, :, None]

    def coef(i):
        return p[:, :, i][:, :, None, None]

    k = coef(6) * X + coef(7) * Y + one
    in_x = ((coef(0) * X + coef(1) * Y) + coef(2)) / k
    in_y = ((coef(3) * X + coef(4) * Y) + coef(5)) / k
    ix = np.round(in_x).astype(np.int32)
    iy = np.round(in_y).astype(np.int32)
    valid = (ix >= 0) & (ix < W) & (iy >= 0) & (iy < H)
    ixc = np.clip(ix, 0, W - 1)
    iyc = np.clip(iy, 0, H - 1)
    flat = (iyc.astype(np.int64) * W + ixc).reshape(B, N, NPIX)

    ys = np.minimum(np.arange(H) * HM // H, HM - 1)
    xs = np.minimum(np.arange(W) * WM // W, WM - 1)
    m_up = mask[:, :, ys][:, :, :, xs]
    wgt = np.where(valid, m_up, np.float32(0.0)).astype(np.float32).reshape(B, N, NPIX)
    return flat, wgt, valid.reshape(B, N, NPIX)


def _build_program(nfix):
    import os

    import concourse.bass as bass
    import concourse.tile as tile
    from concourse import bacc, mybir
    from concourse.ap import AP

    skip_dve = os.environ.get("K_SKIP_DVE") == "1"
    no_fix = os.environ.get("K_NO_FIX") == "1"
    nqueues = int(os.environ.get("K_QUEUES", "4"))
    nc = bacc.Bacc(
        "TRN2",
        target_bir_lowering=False,
        debug=False,
        num_swdge_queues=nqueues,
        dynamic_dma_scratch_size=int(os.environ.get("K_SCRATCH", "32768")),
    )
    xbs = nc.dram_tensor(
        "xbs", [NPIX // 2 * 2 + 1, 2 * C], mybir.dt.bfloat16, kind="ExternalInput"
    ).ap()
    midx = nc.dram_tensor(
        "midx", [NBLK, 128, N * (PAIRS // 16)], mybir.dt.int16, kind="ExternalInput"
    ).ap()
    mw = nc.dram_tensor(
        "mw", [NBLK, 128, N * (PAIRS // 128) * 2], mybir.dt.bfloat16,
        kind="ExternalInput",
    ).ap()
    outd = nc.dram_tensor(
        "outd", [NBLK, 128, PAIRS // 128, 2, C], mybir.dt.bfloat16,
        kind="ExternalOutput",
    ).ap()
    if nfix:
        fidx = nc.dram_tensor(
            "fidx", [nfix, 128, FIXSLOT // 16], mybir.dt.int16, kind="ExternalInput"
        ).ap()
        fw = nc.dram_tensor(
            "fw", [nfix, 128, FIXSLOT // 128], mybir.dt.bfloat16,
            kind="ExternalInput",
        ).ap()
        fixout = nc.dram_tensor(
            "fixout", [nfix, 128, FIXSLOT // 128, C], mybir.dt.bfloat16,
            kind="ExternalOutput",
        ).ap()

    # gather window over 256B units (2 bf16 px): unit MAIN_BASE+k,
    # k in [-32768, 32767] -> units 1..65536 (unit 65536 = sentinel pair).
    srcw = xbs[MAIN_BASE:, :]
    srcf = xbs[FIX_BASE:, :]

    MCOL = PAIRS // 128  # 8
    FCOL = FIXSLOT // 128  # 8
    IDXF = PAIRS // 16  # 64
    nfix_per_blk = [nfix // NBLK + (1 if b < nfix % NBLK else 0) for b in range(NBLK)]

    with tile.TileContext(nc) as tc:
        nreg = nc.gpsimd.to_reg(PAIRS)
        nreg512 = nc.gpsimd.to_reg(PAIRS // 2)
        qctr = [0]
        fctr = [0]
        with (
            tc.tile_pool(name="ip", bufs=12) as ip,
            tc.tile_pool(name="wp", bufs=12) as wp,
            tc.tile_pool(name="gp", bufs=32) as gp,
            tc.tile_pool(name="tp", bufs=32) as tp,
            tc.tile_pool(name="fip", bufs=12) as fip,
            tc.tile_pool(name="fwp", bufs=12) as fwp,
            tc.tile_pool(name="fgp", bufs=12) as fgp,
            tc.tile_pool(name="fpp", bufs=8) as fpp,
        ):

            def emit_fix(f):
                it = fip.tile([128, FIXSLOT // 16], mybir.dt.int16, tag="fi")
                nc.scalar.dma_start(it[:], fidx[f, :, :])
                wt = fwp.tile([128, FCOL], mybir.dt.bfloat16, tag="fw")
                nc.scalar.dma_start(wt[:], fw[f, :, :])
                g = fgp.tile([128, FCOL, 2, C], mybir.dt.bfloat16, tag="fg")
                nc.gpsimd.dma_gather(
                    g[:, :, :, :].rearrange("p c s e -> p c (s e)"),
                    srcf,
                    it[:, :],
                    num_idxs=FIXSLOT,
                    num_idxs_reg=nreg,
                    elem_size=2 * C,
                    queue_num=qctr[0] % nqueues,
                )
                qctr[0] += 1
                pf = fpp.tile([128, FCOL, C], mybir.dt.bfloat16, tag="pf")
                nc.vector.tensor_tensor(
                    pf[:, :, :],
                    g[:, :, 0, :],
                    wt[:, :].broadcast_to([128, FCOL, C]),
                    op=mybir.AluOpType.mult,
                )
                nc.sync.dma_start(fixout[f, :, :, :], pf[:, :, :])

            for blk in range(NBLK):
                it = ip.tile([128, N * IDXF], mybir.dt.int16, tag="mi")
                nc.scalar.dma_start(it[:], midx[blk, :, :])
                wt = wp.tile([128, N, MCOL, 2], mybir.dt.bfloat16, tag="mw")
                nc.scalar.dma_start(
                    wt[:].rearrange("p n c s -> p (n c s)"), mw[blk, :, :]
                )
                # per-n masked products in bf16, then a 9-op pairwise max tree
                gs_last = [None]
                pend = []  # partial maxes awaiting pairing, with tree level
                for n in range(N):
                    g = gp.tile([128, MCOL, 2, C], mybir.dt.bfloat16, tag="g")
                    nc.gpsimd.dma_gather(
                        g[:, :, :, :].rearrange("p c s e -> p c (s e)"),
                        srcw,
                        it[:, n * IDXF : (n + 1) * IDXF],
                        num_idxs=PAIRS,
                        num_idxs_reg=nreg,
                        elem_size=2 * C,
                        queue_num=qctr[0] % nqueues,
                    )
                    qctr[0] += 1
                    if skip_dve:
                        gs_last[0] = g
                        continue
                    t = tp.tile([128, MCOL, 2, C], mybir.dt.bfloat16, tag="t")
                    nc.vector.tensor_tensor(
                        t[:, :, :, :],
                        g[:, :, :, :],
                        wt[:, n, :, :].broadcast_to([128, MCOL, 2, C]),
                        op=mybir.AluOpType.mult,
                    )
                    pend.append((0, t))
                    # greedily combine equal-level partials (keeps tiles hot)
                    while len(pend) >= 2 and pend[-1][0] == pend[-2][0]:
                        l1, a = pend.pop()
                        l0, bb = pend.pop()
                        m = tp.tile([128, MCOL, 2, C], mybir.dt.bfloat16, tag="t")
                        nc.vector.tensor_tensor(
                            m[:, :, :, :], a[:, :, :, :], bb[:, :, :, :],
                            op=mybir.AluOpType.max,
                        )
                        pend.append((l0 + 1, m))
                while len(pend) >= 2:
                    _, a = pend.pop()
                    _, bb = pend.pop()
                    m = tp.tile([128, MCOL, 2, C], mybir.dt.bfloat16, tag="t")
                    nc.vector.tensor_tensor(
                        m[:, :, :, :], a[:, :, :, :], bb[:, :, :, :],
                        op=mybir.AluOpType.max,
                    )
                    pend.append((99, m))
                if skip_dve:
                    acc = tp.tile([128, MCOL, 2, C], mybir.dt.bfloat16, tag="t")
                    nc.vector.tensor_tensor(
                        acc[:, :, :, :],
                        gs_last[0][:, :, :, :],
                        gs_last[0][:, :, :, :],
                        op=mybir.AluOpType.max,
                    )
                else:
                    acc = pend[0][1]
                nc.sync.dma_start(outd[blk, :, :, :, :], acc[:, :, :, :])
                if not no_fix:
                    for _ in range(nfix_per_blk[blk]):
                        emit_fix(fctr[0])
                        fctr[0] += 1
    nc.compile()
    return nc


def _get_program(nfix):
    if nfix not in _programs:
        _programs[nfix] = _build_program(nfix)
    return _programs[nfix]


def _wrap_idx(idx_slots):
    """[..., S] slot-ordered int16 -> [..., 128, S//16] wrapped+replicated."""
    S = idx_slots.shape[-1]
    q = np.arange(S)
    out = np.zeros(idx_slots.shape[:-1] + (128, S // 16), np.int16)
    for g_ in range(8):
        out[..., 16 * g_ + (q % 16), q // 16] = idx_slots
    return out


def kernel(x, transforms, mask):
    from concourse.bass_utils import run_bass_kernel_spmd
    import ml_dtypes

    bf16 = ml_dtypes.bfloat16
    x = np.asarray(x, dtype=np.float32)
    transforms = np.asarray(transforms, dtype=np.float32)
    mask = np.asarray(mask, dtype=np.float32)

    flat, wgt, valid = _host_index_weights(transforms, mask)

    SENT = np.float32(-1e30)
    q = np.arange(PAIRS)

    def unit_of(px):
        # pixel index -> 256B unit in the dual-parity bf16 image
        return np.where(px % 2 == 0, px // 2, 32768 + (px - 1) // 2)

    per_core = []
    max_fix = 0
    for core in range(8):
        b, half = divmod(core, 2)
        sl = slice(half * HALF, (half + 1) * HALF)
        s = flat[b, :, sl]  # [N, HALF]
        w = wgt[b, :, sl]
        v = valid[b, :, sl]

        s0, s1 = s[:, 0::2], s[:, 1::2]
        w0, w1 = w[:, 0::2], w[:, 1::2]
        v0, v1 = v[:, 0::2], v[:, 1::2]

        anchor = np.where(v0, s0, s1 - 1)
        both = v0 & v1
        anyv = v0 | v1
        clean = np.where(both, s1 == s0 + 1, True) & np.where(
            anyv, anchor >= 1, True
        )
        dirty = ~clean
        # trailing-negative guard: last slot of each instruction -> sentinel
        dirty[:, PAIRS - 1 :: PAIRS] = True

        aunit = unit_of(np.maximum(anchor, 1))
        idx = np.where(
            dirty, SENT_IDX, np.where(anyv, aunit - MAIN_BASE, 0)
        ).astype(np.int16)
        mw0 = np.where(dirty, 1.0, np.where(v0, w0, 0.0)).astype(np.float32)
        mw1 = np.where(dirty, 1.0, np.where(v1, w1, 0.0)).astype(np.float32)

        dn, dp = np.nonzero(dirty)
        jj0 = 2 * dp
        m = len(dn)
        fix_j = np.empty(2 * m, np.int64)
        fix_src = np.empty(2 * m, np.int64)
        fix_w = np.empty(2 * m, np.float32)
        fix_j[0::2] = jj0
        fix_j[1::2] = jj0 + 1
        fix_src[0::2] = s0[dn, dp]
        fix_src[1::2] = s1[dn, dp]
        fix_w[0::2] = np.where(v0[dn, dp], w0[dn, dp], 0.0)
        fix_w[1::2] = np.where(v1[dn, dp], w1[dn, dp], 0.0)

        per_core.append((b, idx, mw0, mw1, fix_j, fix_src, fix_w))
        nfix_c = -(-len(fix_j) // (FIXSLOT - 1))
        max_fix = max(max_fix, nfix_c)

    nfix = max(8, -(-max_fix // 8) * 8) if max_fix else 8

    in_maps = []
    fix_meta = []
    for core in range(8):
        b, idx, mw0, mw1, fix_j, fix_src, fix_w = per_core[core]

        idx_b = idx.reshape(N, NBLK, PAIRS)
        miw = _wrap_idx(idx_b)
        miw = np.ascontiguousarray(miw.transpose(1, 2, 0, 3)).reshape(
            NBLK, 128, N * (PAIRS // 16)
        )
        wl = np.zeros((N, NBLK, 128, PAIRS // 128, 2), np.float32)
        wl[:, :, q % 128, q // 128, 0] = mw0.reshape(N, NBLK, PAIRS)[:, :, q]
        wl[:, :, q % 128, q // 128, 1] = mw1.reshape(N, NBLK, PAIRS)[:, :, q]
        mwl = (
            np.ascontiguousarray(wl.transpose(1, 2, 0, 3, 4))
            .reshape(NBLK, 128, N * (PAIRS // 128) * 2)
            .astype(bf16)
        )

        # fixup: 1023 real slots/instruction; window based at unit FIX_BASE
        # so every pixel (incl. 0) is sub-0 of its unit. Pads use an odd
        # pixel (unit >= 32768 -> idx >= 0) so the list never ends negative.
        k = len(fix_src)
        real = FIXSLOT - 1
        tot = nfix * real
        fsrc = np.full(tot, 3, np.int64)  # odd pad pixel: idx stays >= 0
        fwv = np.zeros(tot, np.float32)
        fsrc[:k] = fix_src
        fwv[:k] = fix_w
        funit = unit_of(fsrc)
        funit = np.concatenate(
            [funit.reshape(nfix, real), np.full((nfix, 1), FIX_BASE + 1, np.int64)],
            axis=1,
        )
        fwv_p = np.concatenate(
            [fwv.reshape(nfix, real), np.zeros((nfix, 1), np.float32)], axis=1
        )
        fidx16 = (funit - FIX_BASE).astype(np.int16)
        fiw = _wrap_idx(fidx16)
        qf = np.arange(FIXSLOT)
        fwl2 = np.zeros((nfix, 128, FIXSLOT // 128), np.float32)
        fwl2[:, qf % 128, qf // 128] = fwv_p[:, qf]
        fwl2 = fwl2.astype(bf16)

        # dual-parity bf16 image + sentinel unit
        xb = x[b].reshape(NPIX, C).astype(bf16)
        A = xb.reshape(NPIX // 2, 2 * C)
        Bsh = np.empty((NPIX // 2, 2, C), bf16)
        Bsh[:, 0, :] = xb[1:NPIX:2]
        Bsh[:-1, 1, :] = xb[2:NPIX:2]
        Bsh[-1, 1, :] = xb[0]  # pad = pixel 0 (serves s==0 fixups)
        sent = np.full((1, 2 * C), SENT, np.float32).astype(bf16)
        xbs = np.concatenate(
            [A, Bsh.reshape(NPIX // 2, 2 * C), sent], axis=0
        )

        in_maps.append(
            {
                "xbs": xbs,
                "midx": miw,
                "mw": mwl,
                "fidx": fiw,
                "fw": fwl2,
            }
        )
        fix_meta.append((fix_j, k))

    nc = _get_program(nfix)
    res = run_bass_kernel_spmd(nc, in_maps, list(range(8)))

    out = np.empty((B, H, W, C), np.float32)
    qf = np.arange(FIXSLOT)
    for core in range(8):
        b, half = divmod(core, 2)
        o = res.results[core]["outd"]  # [NBLK,128,8,2,C] bf16
        pair_vals = o[:, q % 128, q // 128, :, :]
        half_px = pair_vals.reshape(HALF, C).astype(np.float32)

        fix_j, k = fix_meta[core]
        if k:
            fo = res.results[core]["fixout"]  # [nfix,128,8,C]
            fv = fo[:, qf % 128, qf // 128, :]  # [nfix, 1024, C]
            fvals = fv[:, : FIXSLOT - 1, :].reshape(-1, C)[:k].astype(np.float32)
            np.maximum.at(half_px, fix_j, fvals)

        out[b, half * ROWS : (half + 1) * ROWS] = half_px.reshape(ROWS, W, C)
    return out


# revision 15
# speedup vs baseline: 1.3391x; 1.0431x over previous
"""AffineTransformLayer as a Trainium2 Bass kernel, SPMD over 8 NeuronCores.

Pair-gather architecture over a dual-parity bf16 image.

Measured cost laws (hardware, this device):
- dma_gather costs ~7.5ns per idx SLOT on one Q7 core pair; 4 queue-pairs
  run concurrently; cost is independent of elem size. So fetch PAIRS of
  output pixels per slot (163840 slots/core vs 327680 for per-pixel).
- The DMA engines run ~11-15GB/s each on small-descriptor gathers and were
  ~80% busy moving 84MB/core in f32 - the byte volume co-limits. Storing
  the image in bf16 halves bytes: each 256B descriptor covers 2 pixels.
- DVE: big tensor_tensor ops with a stride-0 broadcast weight AP plus a
  bf16 pairwise max tree beat per-column scalar_tensor_tensor by ~2.5x.

Image layout ("dual parity"): unit u = 256B = 2 bf16 pixels. Units
0..32767 hold pixel pairs (2k, 2k+1); units 32768..65535 hold (2k+1, 2k+2)
(one-pixel shift), so a pair anchored at ANY pixel parity is one unit.
Unit 65536 is a -1e30 sentinel pair; the int16 idx window (base unit
32769) spans units 1..65536. B-tail pad (unit 65535, sub 1) holds pixel
0's value to make pixel-0 fixups addressable.

Correctness of max over transforms:
- main pass: "clean" pairs (sources consecutive, or any pixel invalid)
  contribute w*x[src] with w=0 for invalid pixels (reference gives 0);
  "dirty" pairs (both valid, sources not consecutive, or anchor
  unreachable, or last-slot-of-instruction) fetch the sentinel pair so
  they never win the on-device max tree.
- fixup pass: each dirty pair's two pixels are gathered via their units,
  weighted on device (weight on the right sub-pixel, 0 on the other), and
  streamed out; the host max-merges these products into the main result at
  host-known positions. Exact coverage: every (n, pixel) entry is in
  exactly one of the two passes.
- the last slot of every 1024-idx instruction is forced dirty (sentinel,
  idx 32767 >= 0) because the ucode drops TRAILING negative idxs, which
  would break the DMA-completion semaphore count.

The fixup stream is provisioned (padded) per call to the worst core's
need, rounded to 8 instructions; programs are compiled per provision size
and cached. Output and fixup products return in bf16 (~0.2% rounding;
harness gate is 2e-2) and the host converts/unshuffles.
"""

import sys

sys.path.insert(0, "/opt/trn_rl_repo")

import numpy as np

B, H, W, C = 4, 256, 256, 64
N = 10
HM, WM = 64, 64
NPIX = H * W
HALF = NPIX // 2  # output pixels per core
ROWS = H // 2
NBLK = 16  # main blocks per core
BLKPX = HALF // NBLK  # 2048 output px per block
PAIRS = BLKPX // 2  # 1024 pair slots per main instruction
SENT_ROW = 65536  # first sentinel row in xbs
MAIN_BASE = 32769  # window base pixel for main pass
FIX_BASE = 32768
SENT_IDX = SENT_ROW - MAIN_BASE  # 32767
FIXSLOT = 1024  # px slots per fixup instruction

_programs = {}


def _host_index_weights(transforms, mask):
    """Reference coordinate math, op-for-op in float32.

    Returns flat src [B,N,NPIX] int64, weight [B,N,NPIX] f32 (0 where
    invalid), valid [B,N,NPIX] bool.
    """
    one = np.float32(1.0)
    p = (transforms.astype(np.float32) / one).astype(np.float32)
    X = np.arange(W, dtype=np.float32)[None, None, None, :]
    Y = np.arange(H, dtype=np.float32)[None, None, :, None]

    def coef(i):
        return p[:, :, i][:, :, None, None]

    k = coef(6) * X + coef(7) * Y + one
    in_x = ((coef(0) * X + coef(1) * Y) + coef(2)) / k
    in_y = ((coef(3) * X + coef(4) * Y) + coef(5)) / k
    ix = np.round(in_x).astype(np.int32)
    iy = np.round(in_y).astype(np.int32)
    valid = (ix >= 0) & (ix < W) & (iy >= 0) & (iy < H)
    ixc = np.clip(ix, 0, W - 1)
    iyc = np.clip(iy, 0, H - 1)
    flat = (iyc.astype(np.int64) * W + ixc).reshape(B, N, NPIX)

    ys = np.minimum(np.arange(H) * HM // H, HM - 1)
    xs = np.minimum(np.arange(W) * WM // W, WM - 1)
    m_up = mask[:, :, ys][:, :, :, xs]
    wgt = np.where(valid, m_up, np.float32(0.0)).astype(np.float32).reshape(B, N, NPIX)
    return flat, wgt, valid.reshape(B, N, NPIX)


def _build_program(nfix):
    import os

    import concourse.bass as bass
    import concourse.tile as tile
    from concourse import bacc, mybir
    from concourse.ap import AP

    skip_dve = os.environ.get("K_SKIP_DVE") == "1"
    no_fix = os.environ.get("K_NO_FIX") == "1"
    nqueues = int(os.environ.get("K_QUEUES", "4"))
    nc = bacc.Bacc(
        "TRN2",
        target_bir_lowering=False,
        debug=False,
        num_swdge_queues=nqueues,
        dynamic_dma_scratch_size=int(os.environ.get("K_SCRATCH", "32768")),
    )
    xbs = nc.dram_tensor(
        "xbs", [NPIX // 2 * 2 + 1, 2 * C], mybir.dt.bfloat16, kind="ExternalInput"
    ).ap()
    midx = nc.dram_tensor(
        "midx", [128, NBLK * N * (PAIRS // 16)], mybir.dt.int16,
        kind="ExternalInput",
    ).ap()
    mw = nc.dram_tensor(
        "mw", [128, NBLK * N * (PAIRS // 128) * 2], mybir.dt.bfloat16,
        kind="ExternalInput",
    ).ap()
    outd = nc.dram_tensor(
        "outd", [NBLK, 128, PAIRS // 128, 2, C], mybir.dt.bfloat16,
        kind="ExternalOutput",
    ).ap()
    if nfix:
        fidx = nc.dram_tensor(
            "fidx", [128, nfix * (FIXSLOT // 16)], mybir.dt.int16,
            kind="ExternalInput",
        ).ap()
        fw = nc.dram_tensor(
            "fw", [128, nfix * (FIXSLOT // 128)], mybir.dt.bfloat16,
            kind="ExternalInput",
        ).ap()
        fixout = nc.dram_tensor(
            "fixout", [nfix, 128, FIXSLOT // 128, C], mybir.dt.bfloat16,
            kind="ExternalOutput",
        ).ap()

    # gather window over 256B units (2 bf16 px): unit MAIN_BASE+k,
    # k in [-32768, 32767] -> units 1..65536 (unit 65536 = sentinel pair).
    srcw = xbs[MAIN_BASE:, :]
    srcf = xbs[FIX_BASE:, :]

    MCOL = PAIRS // 128  # 8
    FCOL = FIXSLOT // 128  # 8
    IDXF = PAIRS // 16  # 64
    nfix_per_blk = [nfix // NBLK + (1 if b < nfix % NBLK else 0) for b in range(NBLK)]

    with tile.TileContext(nc) as tc:
        nreg = nc.gpsimd.to_reg(PAIRS)
        nreg512 = nc.gpsimd.to_reg(PAIRS // 2)
        qctr = [0]
        fctr = [0]
        with (
            tc.tile_pool(name="p1", bufs=1) as p1,
            tc.tile_pool(name="p2", bufs=1) as p2,
            tc.tile_pool(name="p3", bufs=1) as p3,
            tc.tile_pool(name="p4", bufs=1) as p4,
            tc.tile_pool(name="gp", bufs=24) as gp,
            tc.tile_pool(name="tp", bufs=20) as tp,
            tc.tile_pool(name="fgp", bufs=8) as fgp,
            tc.tile_pool(name="fpp", bufs=6) as fpp,
        ):
            it_all = p1.tile([128, NBLK * N * IDXF], mybir.dt.int16, tag="ia")
            nc.scalar.dma_start(it_all[:], midx[:, :])
            wt_all = p2.tile(
                [128, NBLK * N * MCOL * 2], mybir.dt.bfloat16, tag="wa"
            )
            nc.scalar.dma_start(wt_all[:], mw[:, :])
            if nfix:
                fi_all = p3.tile(
                    [128, nfix * (FIXSLOT // 16)], mybir.dt.int16, tag="fa"
                )
                nc.scalar.dma_start(fi_all[:], fidx[:, :])
                fw_all = p4.tile(
                    [128, nfix * FCOL], mybir.dt.bfloat16, tag="fb"
                )
                nc.scalar.dma_start(fw_all[:], fw[:, :])

            def emit_fix(f):
                g = fgp.tile([128, FCOL, 2, C], mybir.dt.bfloat16, tag="fg")
                nc.gpsimd.dma_gather(
                    g[:, :, :, :].rearrange("p c s e -> p c (s e)"),
                    srcf,
                    fi_all[:, f * (FIXSLOT // 16) : (f + 1) * (FIXSLOT // 16)],
                    num_idxs=FIXSLOT,
                    num_idxs_reg=nreg,
                    elem_size=2 * C,
                    queue_num=qctr[0] % nqueues,
                )
                qctr[0] += 1
                pf = fpp.tile([128, FCOL, C], mybir.dt.bfloat16, tag="pf")
                nc.vector.tensor_tensor(
                    pf[:, :, :],
                    g[:, :, 0, :],
                    fw_all[:, f * FCOL : (f + 1) * FCOL].broadcast_to(
                        [128, FCOL, C]
                    ),
                    op=mybir.AluOpType.mult,
                )
                nc.sync.dma_start(fixout[f, :, :, :], pf[:, :, :])

            for blk in range(NBLK):
                it = it_all[:, blk * N * IDXF : (blk + 1) * N * IDXF]
                wt = wt_all[
                    :, blk * N * MCOL * 2 : (blk + 1) * N * MCOL * 2
                ].rearrange("p (n c s) -> p n c s", n=N, c=MCOL)
                # per-n masked products in bf16, then a 9-op pairwise max tree
                gs_last = [None]
                pend = []  # partial maxes awaiting pairing, with tree level
                for n in range(N):
                    g = gp.tile([128, MCOL, 2, C], mybir.dt.bfloat16, tag="g")
                    nc.gpsimd.dma_gather(
                        g[:, :, :, :].rearrange("p c s e -> p c (s e)"),
                        srcw,
                        it[:, n * IDXF : (n + 1) * IDXF],
                        num_idxs=PAIRS,
                        num_idxs_reg=nreg,
                        elem_size=2 * C,
                        queue_num=qctr[0] % nqueues,
                    )
                    qctr[0] += 1
                    if skip_dve:
                        gs_last[0] = g
                        continue
                    t = tp.tile([128, MCOL, 2, C], mybir.dt.bfloat16, tag="t")
                    nc.vector.tensor_tensor(
                        t[:, :, :, :],
                        g[:, :, :, :],
                        wt[:, n, :, :].broadcast_to([128, MCOL, 2, C]),
                        op=mybir.AluOpType.mult,
                    )
                    pend.append((0, t))
                    # greedily combine equal-level partials (keeps tiles hot)
                    while len(pend) >= 2 and pend[-1][0] == pend[-2][0]:
                        l1, a = pend.pop()
                        l0, bb = pend.pop()
                        m = tp.tile([128, MCOL, 2, C], mybir.dt.bfloat16, tag="t")
                        nc.vector.tensor_tensor(
                            m[:, :, :, :], a[:, :, :, :], bb[:, :, :, :],
                            op=mybir.AluOpType.max,
                        )
                        pend.append((l0 + 1, m))
                while len(pend) >= 2:
                    _, a = pend.pop()
                    _, bb = pend.pop()
                    m = tp.tile([128, MCOL, 2, C], mybir.dt.bfloat16, tag="t")
                    nc.vector.tensor_tensor(
                        m[:, :, :, :], a[:, :, :, :], bb[:, :, :, :],
                        op=mybir.AluOpType.max,
                    )
                    pend.append((99, m))
                if skip_dve:
                    acc = tp.tile([128, MCOL, 2, C], mybir.dt.bfloat16, tag="t")
                    nc.vector.tensor_tensor(
                        acc[:, :, :, :],
                        gs_last[0][:, :, :, :],
                        gs_last[0][:, :, :, :],
                        op=mybir.AluOpType.max,
                    )
                else:
                    acc = pend[0][1]
                nc.sync.dma_start(outd[blk, :, :, :, :], acc[:, :, :, :])
                if not no_fix:
                    for _ in range(nfix_per_blk[blk]):
                        emit_fix(fctr[0])
                        fctr[0] += 1
    nc.compile()
    return nc


def _get_program(nfix):
    if nfix not in _programs:
        _programs[nfix] = _build_program(nfix)
    return _programs[nfix]


def _wrap_idx(idx_slots):
    """[..., S] slot-ordered int16 -> [..., 128, S//16] wrapped+replicated."""
    S = idx_slots.shape[-1]
    q = np.arange(S)
    out = np.zeros(idx_slots.shape[:-1] + (128, S // 16), np.int16)
    for g_ in range(8):
        out[..., 16 * g_ + (q % 16), q // 16] = idx_slots
    return out


def kernel(x, transforms, mask):
    from concourse.bass_utils import run_bass_kernel_spmd
    import ml_dtypes

    bf16 = ml_dtypes.bfloat16
    x = np.asarray(x, dtype=np.float32)
    transforms = np.asarray(transforms, dtype=np.float32)
    mask = np.asarray(mask, dtype=np.float32)

    flat, wgt, valid = _host_index_weights(transforms, mask)

    SENT = np.float32(-1e30)
    q = np.arange(PAIRS)

    def unit_of(px):
        # pixel index -> 256B unit in the dual-parity bf16 image
        return np.where(px % 2 == 0, px // 2, 32768 + (px - 1) // 2)

    per_core = []
    max_fix = 0
    for core in range(8):
        b, half = divmod(core, 2)
        sl = slice(half * HALF, (half + 1) * HALF)
        s = flat[b, :, sl]  # [N, HALF]
        w = wgt[b, :, sl]
        v = valid[b, :, sl]

        s0, s1 = s[:, 0::2], s[:, 1::2]
        w0, w1 = w[:, 0::2], w[:, 1::2]
        v0, v1 = v[:, 0::2], v[:, 1::2]

        anchor = np.where(v0, s0, s1 - 1)
        both = v0 & v1
        anyv = v0 | v1
        clean = np.where(both, s1 == s0 + 1, True) & np.where(
            anyv, anchor >= 1, True
        )
        dirty = ~clean
        # trailing-negative guard: last slot of each instruction -> sentinel
        dirty[:, PAIRS - 1 :: PAIRS] = True

        aunit = unit_of(np.maximum(anchor, 1))
        idx = np.where(
            dirty, SENT_IDX, np.where(anyv, aunit - MAIN_BASE, 0)
        ).astype(np.int16)
        mw0 = np.where(dirty, 1.0, np.where(v0, w0, 0.0)).astype(np.float32)
        mw1 = np.where(dirty, 1.0, np.where(v1, w1, 0.0)).astype(np.float32)

        dn, dp = np.nonzero(dirty)
        jj0 = 2 * dp
        m = len(dn)
        fix_j = np.empty(2 * m, np.int64)
        fix_src = np.empty(2 * m, np.int64)
        fix_w = np.empty(2 * m, np.float32)
        fix_j[0::2] = jj0
        fix_j[1::2] = jj0 + 1
        fix_src[0::2] = s0[dn, dp]
        fix_src[1::2] = s1[dn, dp]
        fix_w[0::2] = np.where(v0[dn, dp], w0[dn, dp], 0.0)
        fix_w[1::2] = np.where(v1[dn, dp], w1[dn, dp], 0.0)

        per_core.append((b, idx, mw0, mw1, fix_j, fix_src, fix_w))
        nfix_c = -(-len(fix_j) // (FIXSLOT - 1))
        max_fix = max(max_fix, nfix_c)

    nfix = max(8, -(-max_fix // 8) * 8) if max_fix else 8

    in_maps = []
    fix_meta = []
    for core in range(8):
        b, idx, mw0, mw1, fix_j, fix_src, fix_w = per_core[core]

        idx_b = idx.reshape(N, NBLK, PAIRS)
        miw = _wrap_idx(idx_b)
        miw = np.ascontiguousarray(miw.transpose(1, 2, 0, 3)).reshape(
            NBLK, 128, N * (PAIRS // 16)
        )
        wl = np.zeros((N, NBLK, 128, PAIRS // 128, 2), np.float32)
        wl[:, :, q % 128, q // 128, 0] = mw0.reshape(N, NBLK, PAIRS)[:, :, q]
        wl[:, :, q % 128, q // 128, 1] = mw1.reshape(N, NBLK, PAIRS)[:, :, q]
        mwl = (
            np.ascontiguousarray(wl.transpose(1, 2, 0, 3, 4))
            .reshape(NBLK, 128, N * (PAIRS // 128) * 2)
            .astype(bf16)
        )

        # fixup: 1023 real slots/instruction; window based at unit FIX_BASE
        # so every pixel (incl. 0) is sub-0 of its unit. Pads use an odd
        # pixel (unit >= 32768 -> idx >= 0) so the list never ends negative.
        k = len(fix_src)
        real = FIXSLOT - 1
        tot = nfix * real
        fsrc = np.full(tot, 3, np.int64)  # odd pad pixel: idx stays >= 0
        fwv = np.zeros(tot, np.float32)
        fsrc[:k] = fix_src
        fwv[:k] = fix_w
        funit = unit_of(fsrc)
        funit = np.concatenate(
            [funit.reshape(nfix, real), np.full((nfix, 1), FIX_BASE + 1, np.int64)],
            axis=1,
        )
        fwv_p = np.concatenate(
            [fwv.reshape(nfix, real), np.zeros((nfix, 1), np.float32)], axis=1
        )
        fidx16 = (funit - FIX_BASE).astype(np.int16)
        fiw = _wrap_idx(fidx16)
        qf = np.arange(FIXSLOT)
        fwl2 = np.zeros((nfix, 128, FIXSLOT // 128), np.float32)
        fwl2[:, qf % 128, qf // 128] = fwv_p[:, qf]
        fwl2 = fwl2.astype(bf16)

        # dual-parity bf16 image + sentinel unit
        xb = x[b].reshape(NPIX, C).astype(bf16)
        A = xb.reshape(NPIX // 2, 2 * C)
        Bsh = np.empty((NPIX // 2, 2, C), bf16)
        Bsh[:, 0, :] = xb[1:NPIX:2]
        Bsh[:-1, 1, :] = xb[2:NPIX:2]
        Bsh[-1, 1, :] = xb[0]  # pad = pixel 0 (serves s==0 fixups)
        sent = np.full((1, 2 * C), SENT, np.float32).astype(bf16)
        xbs = np.concatenate(
            [A, Bsh.reshape(NPIX // 2, 2 * C), sent], axis=0
        )

        in_maps.append(
            {
                "xbs": xbs,
                "midx": np.ascontiguousarray(miw.transpose(1, 0, 2)).reshape(
                    128, -1
                ),
                "mw": np.ascontiguousarray(mwl.transpose(1, 0, 2)).reshape(
                    128, -1
                ),
                "fidx": np.ascontiguousarray(fiw.transpose(1, 0, 2)).reshape(
                    128, -1
                ),
                "fw": np.ascontiguousarray(fwl2.transpose(1, 0, 2)).reshape(
                    128, -1
                ),
            }
        )
        fix_meta.append((fix_j, k))

    nc = _get_program(nfix)
    res = run_bass_kernel_spmd(nc, in_maps, list(range(8)))

    out = np.empty((B, H, W, C), np.float32)
    qf = np.arange(FIXSLOT)
    for core in range(8):
        b, half = divmod(core, 2)
        o = res.results[core]["outd"]  # [NBLK,128,8,2,C] bf16
        pair_vals = o[:, q % 128, q // 128, :, :]
        half_px = pair_vals.reshape(HALF, C).astype(np.float32)

        fix_j, k = fix_meta[core]
        if k:
            fo = res.results[core]["fixout"]  # [nfix,128,8,C]
            fv = fo[:, qf % 128, qf // 128, :]  # [nfix, 1024, C]
            fvals = fv[:, : FIXSLOT - 1, :].reshape(-1, C)[:k].astype(np.float32)
            np.maximum.at(half_px, fix_j, fvals)

        out[b, half * ROWS : (half + 1) * ROWS] = half_px.reshape(ROWS, W, C)
    return out
